# revision 36
# baseline (speedup 1.0000x reference)
"""DIN-style attention layer (B=2048, T=200, D=128) on 8 TRN2 NeuronCores.

Data-parallel: batch is sharded 256 per core; MLP params replicated.

Per-core layout strategy (heavy tensors enter the device in the layout the
TensorEngine needs, in bf16, so there are no on-device transposes or
conversions of big data):
  - key_t2[s] = concat(key[2s].T, key[2s+1].T)  -> [D, 2T] bf16 supertiles
  - val_q[q]  = val[4q:4q+4] interleaved        -> [T, 4D] bf16 quads
  - query passed transposed [D, B] (f32 for per-partition scalars + bf16
    for the Q0 matmul); mask passed as additive f32 (0 keep / MASK_PAD drop)

Math (per batch item b):
  [q, k, q-k, q*k] @ W0  ==  k @ (W0b-W0c) + (q*k) @ W0d + q @ (W0a+W0c)
  Layer 0: two K=128 bf16 matmuls per [D, 2T] supertile (N=400, 1 row/cyc),
  with the q-term + b0 folded into the PReLU bias (per-batch bias column).
  Layer 1: one K=128 matmul per supertile into a [64, 400] PSUM; PReLU1
  writes the two supertiles of a pair into halves of one [128, 400] tile.
  Logits: Wout one-hot-pair stationaries accumulate rows of a [GRP, 2T]
  PSUM tile, so softmax runs batched with batch on partitions.
  Output: attn columns are packed into block-one-hot stationaries
  [T-chunk, 64] (4 batch items per 64-col slab); 16 accumulating N=512
  matmuls produce the whole group's output, batch-major, in one PSUM bank.
"""

import os
import sys

import numpy as np

sys.path.insert(0, "/opt/trn_rl_repo")

import ml_dtypes  # noqa: E402

import concourse.bass as bass  # noqa: E402
import concourse.tile as tile  # noqa: E402
from concourse import bacc, mybir  # noqa: E402
from concourse.bass_utils import run_bass_kernel_spmd  # noqa: E402

f32 = mybir.dt.float32
bf16 = mybir.dt.bfloat16
AF = mybir.ActivationFunctionType
ALU = mybir.AluOpType
bfnp = ml_dtypes.bfloat16

B, T, D, H1, H2 = 2048, 200, 128, 128, 64
NCORES = 8
Bs = B // NCORES            # 256 batch items per core
NSUP = Bs // 2              # 128 supertiles (2 batch items each)
GRP = 16                    # supertiles per softmax group (32 batch items)
NG = NSUP // GRP            # 8 groups
NQ = GRP // 2               # quads per group (8)
T2 = 2 * T                  # 400
GB = 2 * GRP                # batch items per group (32)
MASK_PAD = -4294967295.0

_cache = {}
_last_exec_time_ns = None
_last_results = None


def _install_trace_hook():
    """Recreate the NTFF profile hook that bass_utils expects under axon."""
    import contextlib
    import ctypes
    import types

    if "antenv.axon_hooks" in sys.modules:
        return
    so = "/opt/axon/libaxon_pjrt.so"
    try:
        lib = ctypes.CDLL(so)
    except OSError:
        return
    if not hasattr(lib, "axon_start_nrt_profile"):
        return
    lib.axon_start_nrt_profile.argtypes = [ctypes.POINTER(ctypes.c_int64), ctypes.c_size_t]
    lib.axon_start_nrt_profile.restype = ctypes.c_int64
    lib.axon_stop_nrt_profile.argtypes = [ctypes.c_char_p]
    lib.axon_stop_nrt_profile.restype = ctypes.c_int64

    @contextlib.contextmanager
    def _hook(output_dir, device_ids):
        import jax

        jax.devices()
        if device_ids:
            ids = (ctypes.c_int64 * len(device_ids))(*device_ids)
            rc = lib.axon_start_nrt_profile(ids, len(device_ids))
        else:
            rc = lib.axon_start_nrt_profile(None, 0)
        if rc != 0:
            raise RuntimeError(f"axon_start_nrt_profile rc={rc}")
        try:
            yield
        finally:
            n = lib.axon_stop_nrt_profile(str(output_dir).encode())
            print(f"profile: {n} file(s) written to {output_dir}", file=sys.stderr)

    mod = types.ModuleType("antenv.axon_hooks")
    hook = _hook
    mod.get_axon_ntff_profile_hook = lambda: hook
    mod.set_axon_ntff_profile_hook = lambda h: None
    sys.modules["antenv.axon_hooks"] = mod
    from concourse import bass_utils

    bass_utils.upload_artifacts = lambda tmpdir: f"file://{tmpdir}"


def _build(alpha_const: bool):
    nc = bacc.Bacc("TRN2", target_bir_lowering=False, debug=False, num_devices=NCORES)

    def din(name, shape, dt=f32):
        return nc.dram_tensor(name, shape, dt, kind="ExternalInput").ap()

    key_t2 = din("key_t2", [NSUP, D, T2], bf16)
    val_q = din("val_q", [Bs // 4, 128, 8 * D], bf16)
    q_t = din("q_t", [D, Bs])
    q_tb = din("q_tb", [D, Bs], bf16)
    maskadd = din("maskadd", [Bs, T])
    w0k = din("w0k", [D, H1], bf16)
    w0d = din("w0d", [D, H1], bf16)
    w0q = din("w0q", [D, H1], bf16)
    w1 = din("w1", [H1, H2], bf16)
    woh = din("woh", [H1, 8 * GRP], bf16)   # 8 one-hot pair variants of [H1, GRP]
    selv = din("selv", [GB, GRP * T2], bf16)  # per-supertile bias selection rows
    b0c = din("b0c", [H1, 1])
    b1c = din("b1c", [128, 1])
    id32 = din("id32", [32, 32], bf16)
    if alpha_const:
        a0c = din("a0c", [H1, 1])
        a1c = din("a1c", [128, 1])
    else:
        a0q = din("a0q", [H1, 1024])
        a1tp = din("a1tp", [128, T2])
    out = nc.dram_tensor("out", [Bs, D], f32, kind="ExternalOutput").ap()

    with tile.TileContext(nc) as tc:
        from contextlib import ExitStack

        with ExitStack() as ctx:
            const = ctx.enter_context(tc.tile_pool(name="const", bufs=1))
            kqp = ctx.enter_context(tc.tile_pool(name="kq", bufs=12))
            vp = ctx.enter_context(tc.tile_pool(name="v", bufs=NQ + 6))
            h0p = ctx.enter_context(tc.tile_pool(name="h0", bufs=4))
            h1p = ctx.enter_context(tc.tile_pool(name="h1", bufs=3))
            gp = ctx.enter_context(tc.tile_pool(name="grp", bufs=2))
            ps_h0 = ctx.enter_context(tc.tile_pool(name="psh0", bufs=2, space="PSUM"))
            ps_h1 = ctx.enter_context(tc.tile_pool(name="psh1", bufs=1, space="PSUM"))
            ps_lg = ctx.enter_context(tc.tile_pool(name="pslg", bufs=2, space="PSUM"))
            ps_tl = ctx.enter_context(tc.tile_pool(name="pstl", bufs=1, space="PSUM"))

            def cload(ap_in, shape, dtype, name):
                t = const.tile(shape, dtype, tag=name)
                nc.sync.dma_start(t[:], ap_in)
                return t

            w0k_s = cload(w0k, [D, H1], bf16, "w0k")
            w0d_s = cload(w0d, [D, H1], bf16, "w0d")
            w0q_s = cload(w0q, [D, H1], bf16, "w0q")
            w1_s = cload(w1, [H1, H2], bf16, "w1")
            woh_s = cload(woh, [H1, 8 * GRP], bf16, "woh")
            selv_s = cload(selv, [GB, GRP * T2], bf16, "selv")
            qt_s = cload(q_t, [D, Bs], f32, "qts")
            qt_b = cload(q_tb, [D, Bs], bf16, "qtb")
            b0c_s = cload(b0c, [H1, 1], f32, "b0c")
            b1c_s = cload(b1c, [128, 1], f32, "b1c")
            id32_s = cload(id32, [32, 32], bf16, "id32")
            if alpha_const:
                a0c_s = cload(a0c, [H1, 1], f32, "a0c")
                a1c_s = cload(a1c, [128, 1], f32, "a1c")
            else:
                a0q_s = cload(a0q, [H1, 1024], f32, "a0q")
                a1tp_s = cload(a1tp, [128, T2], f32, "a1tp")

            def prelu0(dst_ap, src_ap):
                """dst [128, 1024] = PReLU(src + b0) (bias folded via sel-MM)."""
                if alpha_const:
                    nc.scalar.activation(dst_ap, src_ap, AF.Prelu, bias=b0c_s[:],
                                         scale=1.0, alpha=a0c_s[:])
                else:
                    xs = gp.tile([H1, 1024], f32, tag="fb_x")
                    nc.scalar.activation(xs[:], src_ap, AF.Identity, bias=b0c_s[:])
                    pos = gp.tile([H1, 1024], f32, tag="fb_p")
                    nc.scalar.activation(pos[:], xs[:], AF.Relu)
                    neg = gp.tile([H1, 1024], f32, tag="fb_n")
                    nc.vector.tensor_sub(neg[:], xs[:], pos[:])
                    nega = gp.tile([H1, 1024], f32, tag="fb_na")
                    nc.vector.tensor_mul(nega[:], neg[:], a0q_s[:])
                    nc.vector.tensor_add(dst_ap, pos[:], nega[:])

            def prelu1(dst_ap, src_ap):
                """dst ([128, T2], pair-stacked) = PReLU(src + b1)."""
                if alpha_const:
                    nc.scalar.activation(dst_ap, src_ap, AF.Prelu, bias=b1c_s[:],
                                         scale=1.0, alpha=a1c_s[:])
                else:
                    xs = gp.tile([128, T2], f32, tag="fb1_x")
                    nc.scalar.activation(xs[:], src_ap, AF.Identity, bias=b1c_s[:])
                    pos = gp.tile([128, T2], f32, tag="fb1_p")
                    nc.scalar.activation(pos[:], xs[:], AF.Relu)
                    neg = gp.tile([128, T2], f32, tag="fb1_n")
                    nc.vector.tensor_sub(neg[:], xs[:], pos[:])
                    nega = gp.tile([128, T2], f32, tag="fb1_na")
                    nc.vector.tensor_mul(nega[:], neg[:], a1tp_s[:])
                    nc.vector.tensor_add(dst_ap, pos[:], nega[:])

            # warm-up: keep the PE busy during the initial DMA fill so
            # HAM reaches K=8/8 before the first real matmul
            wps = ps_tl.tile([128, 256], f32, tag="tail", name="warm")
            for r in range(20):
                nc.tensor.matmul(wps[:], w0k_s[:], qt_b[:, 0:Bs],
                                 start=True, stop=True, skip_group_check=True)

            def emit_group_head(g):
                """Mask DMA + Q0 bias columns + logits psum for group g."""
                b_lo = GB * g
                mk = gp.tile([GRP, T2], f32, tag="mask")
                nc.sync.dma_start(
                    mk[:].rearrange("s (two t) -> s two t", two=2),
                    maskadd[b_lo:b_lo + GB].rearrange("(s two) t -> s two t", two=2),
                )
                # Q0T = q_grp.T @ (W0a+W0c): per-batch bias rows, fed back
                # into layer 0 via constant selection-matrix matmuls
                q0ps = ps_tl.tile([GB, D], f32, tag="tail")
                nc.tensor.matmul(q0ps[:], qt_b[:, b_lo:b_lo + GB], w0q_s[:],
                                 start=True, stop=True)
                q0bT = gp.tile([GB, D], bf16, tag="q0bT")
                nc.vector.tensor_copy(q0bT[:], q0ps[:])
                lg = ps_lg.tile([GRP, T2], f32, tag="lg")
                return {"lg": lg, "mk": mk, "q0bT": q0bT, "vtiles": [], "b_lo": b_lo}

            # Deferred PE ops: the TensorEngine executes its stream in order,
            # so a matmul that waits on ACT output (L1 on PReLU0, logits on
            # PReLU1) must sit LATER in the stream than independent work.
            # Each supertile queues its dependent matmuls to be emitted while
            # the NEXT supertile's L0 work is already in flight.
            deferred = []

            def flush_deferred():
                for f in deferred:
                    f()
                deferred.clear()

            def emit_pair(g, u, st):
                lg, q0bT = st["lg"], st["q0bT"]
                h0d = ps_h0.tile([128, 1024], f32, tag="h0d")
                h1ps = ps_h1.tile([128, T2], f32, tag="h1ps")
                h1t = h1p.tile([128, T2], bf16, tag="h1")
                # one quad (4 batch items) of val per pair of supertiles,
                # host-packed as [128, 1024]: chunk A in cols 0:512, chunk B
                # (t=128..199, zero-padded) in cols 512:1024
                qg = NQ * g + u
                vq = vp.tile([128, 8 * D], bf16, tag="vq")
                nc.gpsimd.dma_start(vq[:], val_q[qg])
                st["vtiles"].append((vq[:, 0:4 * D], vq[0:72, 4 * D:8 * D]))
                s0 = GRP * g + 2 * u
                kt2 = kqp.tile([D, 2 * T2], bf16, tag="kt")
                nc.sync.dma_start(
                    kt2[:].rearrange("d (two t) -> d two t", two=2),
                    key_t2[s0:s0 + 2].rearrange("two d t -> d two t"),
                )
                for j in (0, 1):
                    sl = 2 * u + j
                    s = s0 + j
                    kt = kt2[:, T2 * j:T2 * (j + 1)]
                    qk = kqp.tile([D, T2], bf16, tag="qk")
                    nc.vector.tensor_scalar_mul(qk[:, 0:T], kt[:, 0:T],
                                                qt_s[:, 2 * s:2 * s + 1])
                    nc.vector.tensor_scalar_mul(qk[:, T:T2], kt[:, T:T2],
                                                qt_s[:, 2 * s + 1:2 * s + 2])

                    dst = h0d[:, 512 * j:512 * j + T2]
                    nc.tensor.matmul(dst, w0k_s[:], kt, start=True, stop=False)
                    nc.tensor.matmul(dst, w0d_s[:], qk[:], start=False, stop=False)
                    nc.tensor.matmul(dst, q0bT[:],
                                     selv_s[:, T2 * sl:T2 * (sl + 1)],
                                     start=False, stop=True)

                pend = list(deferred)
                deferred.clear()

                h0t = h0p.tile([H1, 1024], bf16, tag="h0t")
                prelu0(h0t[:], h0d[:])

                # emit matmuls deferred by the previous pair now that this
                # pair's six L0 matmuls are queued ahead of them
                for f in pend:
                    f()

                def l1(h0t=h0t, h1ps=h1ps, h1t=h1t, u=u, lg=lg):
                    for j in (0, 1):
                        nc.tensor.matmul(h1ps[64 * j:64 * j + 64, :], w1_s[:],
                                         h0t[:, 512 * j:512 * j + T2],
                                         start=True, stop=True)
                    prelu1(h1t[:], h1ps[:])

                    def logits():
                        nc.tensor.matmul(
                            lg[:], woh_s[:, GRP * u:GRP * (u + 1)], h1t[:],
                            start=(u == 0), stop=(u == GRP // 2 - 1),
                            skip_group_check=True)
                    deferred.append(logits)
                deferred.append(l1)

            def emit_tail(st):
                # --- batched softmax over the group: [GRP, 2, T] ---
                lg, mk, vtiles, b_lo = st["lg"], st["mk"], st["vtiles"], st["b_lo"]
                lsb = gp.tile([GRP, T2], f32, tag="lsb")
                nc.vector.tensor_add(lsb[:], lg[:], mk[:])
                nm = gp.tile([GRP, 2], f32, tag="nm")
                nc.vector.tensor_reduce(nm[:], lsb[:].rearrange("s (two t) -> s two t", two=2),
                                        mybir.AxisListType.X, ALU.max, negate=True)
                ae = gp.tile([GRP, T2], f32, tag="ae")
                sums = gp.tile([GRP, 2], f32, tag="sums")
                for half in (0, 1):
                    nc.scalar.activation(ae[:, half * T:(half + 1) * T],
                                         lsb[:, half * T:(half + 1) * T], AF.Exp,
                                         bias=nm[:, half:half + 1], scale=1.0,
                                         accum_out=sums[:, half:half + 1])
                inv = gp.tile([GRP, 2], f32, tag="inv")
                nc.vector.reciprocal(inv[:], sums[:])
                an = gp.tile([GRP, T2], bf16, tag="an")
                nc.vector.tensor_scalar_mul(an[:, 0:T], ae[:, 0:T], inv[:, 0:1])
                nc.vector.tensor_scalar_mul(an[:, T:T2], ae[:, T:T2], inv[:, 1:2])

                # transpose attn into [T-chunk, b] layout: 4 blocks of GRP cols
                # (block 0/1 = even/odd b, t 0:128; block 2/3 = even/odd, t 128:200)
                atps = ps_tl.tile([128, 4 * GRP], bf16, tag="tail")
                idg = id32_s[0:GRP, 0:GRP]
                nc.tensor.transpose(atps[0:128, 0:GRP], an[:, 0:128], idg)
                nc.tensor.transpose(atps[0:128, GRP:2 * GRP], an[:, T:T + 128], idg)
                nc.tensor.transpose(atps[0:72, 2 * GRP:3 * GRP], an[:, 128:T], idg)
                nc.tensor.transpose(atps[0:72, 3 * GRP:4 * GRP], an[:, T + 128:T2], idg)
                at_sb = gp.tile([128, 4 * GRP], bf16, tag="at")
                nc.vector.tensor_copy(at_sb[:], atps[:])

                # block-one-hot attn stationaries: slab u (cols 32u..32u+31)
                # holds attn for b = 4u+i at within-slab col (u + 8i); the
                # matmul therefore writes b's output to psum row (u + 8i).
                # Global one-hot col = 33u + 8i -> one strided copy per i.
                ohA = gp.tile([128, 32 * NQ], bf16, tag="ohA")
                nc.vector.memset(ohA[:], 0.0)
                ohB = gp.tile([72, 32 * NQ], bf16, tag="ohB")
                nc.vector.memset(ohB[:], 0.0)
                for i in range(4):
                    # b = 4u+i -> at_sb col (b%2)*GRP + b//2 = (i%2)*GRP + 2u + i//2
                    c0 = (i % 2) * GRP + i // 2
                    sa = at_sb[0:128, c0:c0 + 2 * (NQ - 1) + 1:2]
                    da = ohA[:, 8 * i:8 * i + 33 * (NQ - 1) + 1:33]
                    nc.vector.tensor_copy(da, sa)
                    sb_ = at_sb[0:72, 2 * GRP + c0:2 * GRP + c0 + 2 * (NQ - 1) + 1:2]
                    db = ohB[:, 8 * i:8 * i + 33 * (NQ - 1) + 1:33]
                    nc.vector.tensor_copy(db, sb_)

                # V-step: 16 accumulating N=512 matmuls -> whole group output
                vops = ps_tl.tile([GB, 4 * D], f32, tag="tail")
                for u in range(NQ):
                    vqa, vqb = vtiles[u]
                    nc.tensor.matmul(vops[:], ohA[:, 32 * u:32 * u + 32],
                                     vqa[:], start=(u == 0), stop=False,
                                     skip_group_check=True)
                    nc.tensor.matmul(vops[:], ohB[:, 32 * u:32 * u + 32],
                                     vqb[:], start=False, stop=(u == NQ - 1),
                                     skip_group_check=True)

                # psum row u+8i holds b=4u+i at col-block i
                vsb = gp.tile([GB, 4 * D], f32, tag="vsb")
                nc.vector.tensor_copy(vsb[:], vops[:])
                for i in range(4):
                    nc.gpsimd.dma_start(
                        out[b_lo + i:b_lo + i + 4 * (NQ - 1) + 1:4],
                        vsb[8 * i:8 * i + NQ, 128 * i:128 * (i + 1)],
                    )

            # software pipeline: group g's supertile phase overlaps group
            # g-1's softmax/V tail so the TensorEngine never drains.
            prev = None
            for g in range(NG):
                st = emit_group_head(g)
                for u in range(NQ):
                    emit_pair(g, u, st)
                    if u == 2 and prev is not None:
                        emit_tail(prev)
                prev = st
            flush_deferred()
            flush_deferred()
            emit_tail(prev)

    nc.compile()
    return nc


def _prep_host(inputs):
    """Split/relayout the full inputs into 8 per-core input maps."""
    query = np.ascontiguousarray(inputs["query"], dtype=np.float32)
    key = np.ascontiguousarray(inputs["key"], dtype=np.float32)
    val = np.ascontiguousarray(inputs["val"], dtype=np.float32)
    mask = inputs["mask"]
    W0 = np.asarray(inputs["W0"], dtype=np.float32)
    b0 = np.asarray(inputs["b0"], dtype=np.float32)
    a0 = np.asarray(inputs["a0"], dtype=np.float32)
    W1 = np.asarray(inputs["W1"], dtype=np.float32)
    b1 = np.asarray(inputs["b1"], dtype=np.float32)
    a1 = np.asarray(inputs["a1"], dtype=np.float32)
    Wout = np.asarray(inputs["Wout"], dtype=np.float32)
    # bout shifts every unmasked logit equally -> cancels in softmax; unused.

    alpha_const = bool(np.all(a0 == a0[0:1, :]) and np.all(a1 == a1[0:1, :]))

    w0a, w0b, w0c, w0d = W0[0:D], W0[D:2 * D], W0[2 * D:3 * D], W0[3 * D:4 * D]

    woh = np.zeros((H1, 8 * GRP), dtype=np.float32)
    for u in range(GRP // 2):
        woh[0:H2, GRP * u + 2 * u] = Wout[:, 0]
        woh[H2:2 * H2, GRP * u + 2 * u + 1] = Wout[:, 0]

    # selection rows: variant sl routes Q0T rows (2sl, 2sl+1) onto the two
    # T-halves of the supertile's [H1, 2T] layer-0 psum block
    selv = np.zeros((GB, GRP * T2), dtype=np.float32)
    for sl in range(GRP):
        selv[2 * sl, T2 * sl:T2 * sl + T] = 1.0
        selv[2 * sl + 1, T2 * sl + T:T2 * (sl + 1)] = 1.0

    consts = {
        "w0k": (w0b - w0c).astype(bfnp),
        "w0d": w0d.astype(bfnp),
        "w0q": (w0a + w0c).astype(bfnp),
        "w1": W1.astype(bfnp),
        "woh": woh.astype(bfnp),
        "selv": selv.astype(bfnp),
        "b0c": b0.reshape(H1, 1).copy(),
        "b1c": np.concatenate([b1, b1]).reshape(128, 1),
        "id32": np.eye(32, dtype=np.float32).astype(bfnp),
    }
    if alpha_const:
        consts["a0c"] = a0[0].reshape(H1, 1).copy()
        consts["a1c"] = np.concatenate([a1[0], a1[0]]).reshape(128, 1)
    else:
        a0t = np.ascontiguousarray(a0.T)
        a0q = np.ones((H1, 1024), dtype=np.float32)
        a0q[:, 0:T] = a0t
        a0q[:, T:T2] = a0t
        a0q[:, 512:512 + T] = a0t
        a0q[:, 512 + T:512 + T2] = a0t
        consts["a0q"] = a0q
        a1t = np.ascontiguousarray(a1.T)
        consts["a1tp"] = np.concatenate(
            [np.concatenate([a1t, a1t], axis=1)] * 2, axis=0)

    maskadd_full = np.where(mask == 0, np.float32(MASK_PAD), np.float32(0.0))
    maskadd_full = maskadd_full.astype(np.float32)

    in_maps = []
    for c in range(NCORES):
        sl = slice(c * Bs, (c + 1) * Bs)
        m = dict(consts)
        kb = key[sl].astype(bfnp)                     # [Bs, T, D]
        m["key_t2"] = np.ascontiguousarray(
            kb.reshape(NSUP, 2, T, D).transpose(0, 3, 1, 2)).reshape(NSUP, D, T2)
        vb = val[sl].astype(bfnp)
        vq = vb.reshape(Bs // 4, 4, T, D).transpose(0, 2, 1, 3).reshape(Bs // 4, T, 4 * D)
        vq2 = np.zeros((Bs // 4, 128, 8 * D), dtype=bfnp)
        vq2[:, :, 0:4 * D] = vq[:, 0:128, :]
        vq2[:, 0:72, 4 * D:8 * D] = vq[:, 128:T, :]
        m["val_q"] = vq2
        qs = np.ascontiguousarray(query[sl].T)
        m["q_t"] = qs
        m["q_tb"] = qs.astype(bfnp)
        m["maskadd"] = np.ascontiguousarray(maskadd_full[sl])
        in_maps.append(m)
    return in_maps, alpha_const


def kernel(**inputs) -> np.ndarray:
    global _last_exec_time_ns, _last_results
    in_maps, alpha_const = _prep_host(inputs)

    ck = ("graph", alpha_const)
    if ck not in _cache:
        _cache[ck] = _build(alpha_const)
    nc = _cache[ck]

    trace = bool(os.environ.get("BASS_KERNEL_TRACE"))
    if trace:
        _install_trace_hook()
    res = run_bass_kernel_spmd(nc, in_maps, core_ids=list(range(NCORES)), trace=trace)
    _last_exec_time_ns = res.exec_time_ns
    _last_results = res
    return np.concatenate([res.results[c]["out"] for c in range(NCORES)], axis=0)


# revision 37
# speedup vs baseline: 1.0393x; 1.0393x over previous
"""DIN-style attention layer (B=2048, T=200, D=128) on 8 TRN2 NeuronCores.

Data-parallel: batch is sharded 256 per core; MLP params replicated.

Per-core layout strategy (heavy tensors enter the device in the layout the
TensorEngine needs, in bf16, so there are no on-device transposes or
conversions of big data):
  - key_t2[s] = concat(key[2s].T, key[2s+1].T)  -> [D, 2T] bf16 supertiles
  - val_q[q]  = val[4q:4q+4] interleaved        -> [T, 4D] bf16 quads
  - query passed transposed [D, B] (f32 for per-partition scalars + bf16
    for the Q0 matmul); mask passed as additive f32 (0 keep / MASK_PAD drop)

Math (per batch item b):
  [q, k, q-k, q*k] @ W0  ==  k @ (W0b-W0c) + (q*k) @ W0d + q @ (W0a+W0c)
  Layer 0: two K=128 bf16 matmuls per [D, 2T] supertile (N=400, 1 row/cyc),
  with the q-term + b0 folded into the PReLU bias (per-batch bias column).
  Layer 1: one K=128 matmul per supertile into a [64, 400] PSUM; PReLU1
  writes the two supertiles of a pair into halves of one [128, 400] tile.
  Logits: Wout one-hot-pair stationaries accumulate rows of a [GRP, 2T]
  PSUM tile, so softmax runs batched with batch on partitions.
  Output: attn columns are packed into block-one-hot stationaries
  [T-chunk, 64] (4 batch items per 64-col slab); 16 accumulating N=512
  matmuls produce the whole group's output, batch-major, in one PSUM bank.
"""

import os
import sys

import numpy as np

sys.path.insert(0, "/opt/trn_rl_repo")

import ml_dtypes  # noqa: E402

import concourse.bass as bass  # noqa: E402
import concourse.tile as tile  # noqa: E402
from concourse import bacc, mybir  # noqa: E402
from concourse.bass_utils import run_bass_kernel_spmd  # noqa: E402

f32 = mybir.dt.float32
bf16 = mybir.dt.bfloat16
AF = mybir.ActivationFunctionType
ALU = mybir.AluOpType
bfnp = ml_dtypes.bfloat16

B, T, D, H1, H2 = 2048, 200, 128, 128, 64
NCORES = 8
Bs = B // NCORES            # 256 batch items per core
NSUP = Bs // 2              # 128 supertiles (2 batch items each)
GRP = 16                    # supertiles per softmax group (32 batch items)
NG = NSUP // GRP            # 8 groups
NQ = GRP // 2               # quads per group (8)
T2 = 2 * T                  # 400
GB = 2 * GRP                # batch items per group (32)
MASK_PAD = -4294967295.0

_cache = {}
_last_exec_time_ns = None
_last_results = None


def _install_trace_hook():
    """Recreate the NTFF profile hook that bass_utils expects under axon."""
    import contextlib
    import ctypes
    import types

    if "antenv.axon_hooks" in sys.modules:
        return
    so = "/opt/axon/libaxon_pjrt.so"
    try:
        lib = ctypes.CDLL(so)
    except OSError:
        return
    if not hasattr(lib, "axon_start_nrt_profile"):
        return
    lib.axon_start_nrt_profile.argtypes = [ctypes.POINTER(ctypes.c_int64), ctypes.c_size_t]
    lib.axon_start_nrt_profile.restype = ctypes.c_int64
    lib.axon_stop_nrt_profile.argtypes = [ctypes.c_char_p]
    lib.axon_stop_nrt_profile.restype = ctypes.c_int64

    @contextlib.contextmanager
    def _hook(output_dir, device_ids):
        import jax

        jax.devices()
        if device_ids:
            ids = (ctypes.c_int64 * len(device_ids))(*device_ids)
            rc = lib.axon_start_nrt_profile(ids, len(device_ids))
        else:
            rc = lib.axon_start_nrt_profile(None, 0)
        if rc != 0:
            raise RuntimeError(f"axon_start_nrt_profile rc={rc}")
        try:
            yield
        finally:
            n = lib.axon_stop_nrt_profile(str(output_dir).encode())
            print(f"profile: {n} file(s) written to {output_dir}", file=sys.stderr)

    mod = types.ModuleType("antenv.axon_hooks")
    hook = _hook
    mod.get_axon_ntff_profile_hook = lambda: hook
    mod.set_axon_ntff_profile_hook = lambda h: None
    sys.modules["antenv.axon_hooks"] = mod
    from concourse import bass_utils

    bass_utils.upload_artifacts = lambda tmpdir: f"file://{tmpdir}"


def _build(alpha_const: bool):
    nc = bacc.Bacc("TRN2", target_bir_lowering=False, debug=False, num_devices=NCORES)

    def din(name, shape, dt=f32):
        return nc.dram_tensor(name, shape, dt, kind="ExternalInput").ap()

    key_t2 = din("key_t2", [NSUP, D, T2], bf16)
    val_q = din("val_q", [Bs // 4, T, 4 * D], bf16)
    q_t = din("q_t", [D, Bs])
    q_tb = din("q_tb", [D, Bs], bf16)
    maskadd = din("maskadd", [Bs, T])
    w0k = din("w0k", [D, H1], bf16)
    w0d = din("w0d", [D, H1], bf16)
    w0q = din("w0q", [D, H1], bf16)
    w1 = din("w1", [H1, H2], bf16)
    woh = din("woh", [H1, 8 * GRP], bf16)   # 8 one-hot pair variants of [H1, GRP]
    selv = din("selv", [GB, GRP * T2], bf16)  # per-supertile bias selection rows
    b0c = din("b0c", [H1, 1])
    b1c = din("b1c", [128, 1])
    id32 = din("id32", [32, 32], bf16)
    if alpha_const:
        a0c = din("a0c", [H1, 1])
        a1c = din("a1c", [128, 1])
    else:
        a0q = din("a0q", [H1, 1024])
        a1tp = din("a1tp", [128, T2])
    out = nc.dram_tensor("out", [Bs, D], f32, kind="ExternalOutput").ap()

    with tile.TileContext(nc) as tc:
        from contextlib import ExitStack

        with ExitStack() as ctx:
            const = ctx.enter_context(tc.tile_pool(name="const", bufs=1))
            kqp = ctx.enter_context(tc.tile_pool(name="kq", bufs=12))
            vp = ctx.enter_context(tc.tile_pool(name="v", bufs=NQ + 6))
            h0p = ctx.enter_context(tc.tile_pool(name="h0", bufs=4))
            h1p = ctx.enter_context(tc.tile_pool(name="h1", bufs=3))
            gp = ctx.enter_context(tc.tile_pool(name="grp", bufs=2))
            ps_h0 = ctx.enter_context(tc.tile_pool(name="psh0", bufs=2, space="PSUM"))
            ps_h1 = ctx.enter_context(tc.tile_pool(name="psh1", bufs=1, space="PSUM"))
            ps_lg = ctx.enter_context(tc.tile_pool(name="pslg", bufs=2, space="PSUM"))
            ps_tl = ctx.enter_context(tc.tile_pool(name="pstl", bufs=1, space="PSUM"))

            def cload(ap_in, shape, dtype, name):
                t = const.tile(shape, dtype, tag=name)
                nc.sync.dma_start(t[:], ap_in)
                return t

            w0k_s = cload(w0k, [D, H1], bf16, "w0k")
            w0d_s = cload(w0d, [D, H1], bf16, "w0d")
            w0q_s = cload(w0q, [D, H1], bf16, "w0q")
            w1_s = cload(w1, [H1, H2], bf16, "w1")
            woh_s = cload(woh, [H1, 8 * GRP], bf16, "woh")
            selv_s = cload(selv, [GB, GRP * T2], bf16, "selv")
            qt_s = cload(q_t, [D, Bs], f32, "qts")
            qt_b = cload(q_tb, [D, Bs], bf16, "qtb")
            b0c_s = cload(b0c, [H1, 1], f32, "b0c")
            b1c_s = cload(b1c, [128, 1], f32, "b1c")
            id32_s = cload(id32, [32, 32], bf16, "id32")
            if alpha_const:
                a0c_s = cload(a0c, [H1, 1], f32, "a0c")
                a1c_s = cload(a1c, [128, 1], f32, "a1c")
            else:
                a0q_s = cload(a0q, [H1, 1024], f32, "a0q")
                a1tp_s = cload(a1tp, [128, T2], f32, "a1tp")

            def prelu0(dst_ap, src_ap):
                """dst [128, 1024] = PReLU(src + b0) (bias folded via sel-MM)."""
                if alpha_const:
                    nc.scalar.activation(dst_ap, src_ap, AF.Prelu, bias=b0c_s[:],
                                         scale=1.0, alpha=a0c_s[:])
                else:
                    xs = gp.tile([H1, 1024], f32, tag="fb_x")
                    nc.scalar.activation(xs[:], src_ap, AF.Identity, bias=b0c_s[:])
                    pos = gp.tile([H1, 1024], f32, tag="fb_p")
                    nc.scalar.activation(pos[:], xs[:], AF.Relu)
                    neg = gp.tile([H1, 1024], f32, tag="fb_n")
                    nc.vector.tensor_sub(neg[:], xs[:], pos[:])
                    nega = gp.tile([H1, 1024], f32, tag="fb_na")
                    nc.vector.tensor_mul(nega[:], neg[:], a0q_s[:])
                    nc.vector.tensor_add(dst_ap, pos[:], nega[:])

            def prelu1(dst_ap, src_ap):
                """dst ([128, T2], pair-stacked) = PReLU(src + b1)."""
                if alpha_const:
                    nc.scalar.activation(dst_ap, src_ap, AF.Prelu, bias=b1c_s[:],
                                         scale=1.0, alpha=a1c_s[:])
                else:
                    xs = gp.tile([128, T2], f32, tag="fb1_x")
                    nc.scalar.activation(xs[:], src_ap, AF.Identity, bias=b1c_s[:])
                    pos = gp.tile([128, T2], f32, tag="fb1_p")
                    nc.scalar.activation(pos[:], xs[:], AF.Relu)
                    neg = gp.tile([128, T2], f32, tag="fb1_n")
                    nc.vector.tensor_sub(neg[:], xs[:], pos[:])
                    nega = gp.tile([128, T2], f32, tag="fb1_na")
                    nc.vector.tensor_mul(nega[:], neg[:], a1tp_s[:])
                    nc.vector.tensor_add(dst_ap, pos[:], nega[:])

            # warm-up: keep the PE busy during the initial DMA fill so
            # HAM reaches K=8/8 before the first real matmul
            wps = ps_tl.tile([128, 256], f32, tag="tail", name="warm")
            for r in range(20):
                nc.tensor.matmul(wps[:], w0k_s[:], qt_b[:, 0:Bs],
                                 start=True, stop=True, skip_group_check=True)

            def emit_group_head(g):
                """Mask DMA + Q0 bias columns + logits psum for group g."""
                b_lo = GB * g
                mk = gp.tile([GRP, T2], f32, tag="mask")
                nc.sync.dma_start(
                    mk[:].rearrange("s (two t) -> s two t", two=2),
                    maskadd[b_lo:b_lo + GB].rearrange("(s two) t -> s two t", two=2),
                )
                # Q0T = q_grp.T @ (W0a+W0c): per-batch bias rows, fed back
                # into layer 0 via constant selection-matrix matmuls
                q0ps = ps_tl.tile([GB, D], f32, tag="tail")
                nc.tensor.matmul(q0ps[:], qt_b[:, b_lo:b_lo + GB], w0q_s[:],
                                 start=True, stop=True)
                q0bT = gp.tile([GB, D], bf16, tag="q0bT")
                nc.vector.tensor_copy(q0bT[:], q0ps[:])
                lg = ps_lg.tile([GRP, T2], f32, tag="lg")
                return {"lg": lg, "mk": mk, "q0bT": q0bT, "vtiles": [], "b_lo": b_lo}

            # Deferred PE ops: the TensorEngine executes its stream in order,
            # so a matmul that waits on ACT output (L1 on PReLU0, logits on
            # PReLU1) must sit LATER in the stream than independent work.
            # Each supertile queues its dependent matmuls to be emitted while
            # the NEXT supertile's L0 work is already in flight.
            deferred = []

            def flush_deferred():
                for f in deferred:
                    f()
                deferred.clear()

            def emit_pair(g, u, st):
                lg, q0bT = st["lg"], st["q0bT"]
                h0d = ps_h0.tile([128, 1024], f32, tag="h0d")
                h1ps = ps_h1.tile([128, T2], f32, tag="h1ps")
                h1t = h1p.tile([128, T2], bf16, tag="h1")
                # one quad (4 batch items) of val per pair of supertiles
                qg = NQ * g + u
                vqa = vp.tile([128, 4 * D], bf16, tag="vqa")
                nc.gpsimd.dma_start(vqa[:], val_q[qg, 0:128, :])
                vqb = vp.tile([72, 4 * D], bf16, tag="vqb")
                nc.gpsimd.dma_start(vqb[:], val_q[qg, 128:T, :])
                st["vtiles"].append((vqa, vqb))
                for j in (0, 1):
                    sl = 2 * u + j
                    s = GRP * g + sl
                    kt = kqp.tile([D, T2], bf16, tag="kt")
                    nc.sync.dma_start(kt[:], key_t2[s])
                    qk = kqp.tile([D, T2], bf16, tag="qk")
                    nc.vector.tensor_scalar_mul(qk[:, 0:T], kt[:, 0:T],
                                                qt_s[:, 2 * s:2 * s + 1])
                    nc.vector.tensor_scalar_mul(qk[:, T:T2], kt[:, T:T2],
                                                qt_s[:, 2 * s + 1:2 * s + 2])

                    dst = h0d[:, 512 * j:512 * j + T2]
                    nc.tensor.matmul(dst, w0k_s[:], kt[:], start=True, stop=False)
                    nc.tensor.matmul(dst, w0d_s[:], qk[:], start=False, stop=False)
                    nc.tensor.matmul(dst, q0bT[:],
                                     selv_s[:, T2 * sl:T2 * (sl + 1)],
                                     start=False, stop=True)

                pend = list(deferred)
                deferred.clear()

                h0t = h0p.tile([H1, 1024], bf16, tag="h0t")
                prelu0(h0t[:], h0d[:])

                # emit matmuls deferred by the previous pair now that this
                # pair's six L0 matmuls are queued ahead of them
                for f in pend:
                    f()

                def l1(h0t=h0t, h1ps=h1ps, h1t=h1t, u=u, lg=lg):
                    for j in (0, 1):
                        nc.tensor.matmul(h1ps[64 * j:64 * j + 64, :], w1_s[:],
                                         h0t[:, 512 * j:512 * j + T2],
                                         start=True, stop=True)
                    prelu1(h1t[:], h1ps[:])

                    def logits():
                        nc.tensor.matmul(
                            lg[:], woh_s[:, GRP * u:GRP * (u + 1)], h1t[:],
                            start=(u == 0), stop=(u == GRP // 2 - 1),
                            skip_group_check=True)
                    deferred.append(logits)
                deferred.append(l1)

            def emit_tail(st):
                # --- batched softmax over the group: [GRP, 2, T] ---
                lg, mk, vtiles, b_lo = st["lg"], st["mk"], st["vtiles"], st["b_lo"]
                lsb = gp.tile([GRP, T2], f32, tag="lsb")
                nc.vector.tensor_add(lsb[:], lg[:], mk[:])
                nm = gp.tile([GRP, 2], f32, tag="nm")
                nc.vector.tensor_reduce(nm[:], lsb[:].rearrange("s (two t) -> s two t", two=2),
                                        mybir.AxisListType.X, ALU.max, negate=True)
                ae = gp.tile([GRP, T2], f32, tag="ae")
                sums = gp.tile([GRP, 2], f32, tag="sums")
                for half in (0, 1):
                    nc.scalar.activation(ae[:, half * T:(half + 1) * T],
                                         lsb[:, half * T:(half + 1) * T], AF.Exp,
                                         bias=nm[:, half:half + 1], scale=1.0,
                                         accum_out=sums[:, half:half + 1])
                inv = gp.tile([GRP, 2], f32, tag="inv")
                nc.vector.reciprocal(inv[:], sums[:])
                an = gp.tile([GRP, T2], bf16, tag="an")
                nc.vector.tensor_scalar_mul(an[:, 0:T], ae[:, 0:T], inv[:, 0:1])
                nc.vector.tensor_scalar_mul(an[:, T:T2], ae[:, T:T2], inv[:, 1:2])

                # transpose attn into [T-chunk, b] layout: 4 blocks of GRP cols
                # (block 0/1 = even/odd b, t 0:128; block 2/3 = even/odd, t 128:200)
                atps = ps_tl.tile([128, 4 * GRP], bf16, tag="tail")
                idg = id32_s[0:GRP, 0:GRP]
                nc.tensor.transpose(atps[0:128, 0:GRP], an[:, 0:128], idg)
                nc.tensor.transpose(atps[0:128, GRP:2 * GRP], an[:, T:T + 128], idg)
                nc.tensor.transpose(atps[0:72, 2 * GRP:3 * GRP], an[:, 128:T], idg)
                nc.tensor.transpose(atps[0:72, 3 * GRP:4 * GRP], an[:, T + 128:T2], idg)
                at_sb = gp.tile([128, 4 * GRP], bf16, tag="at")
                nc.vector.tensor_copy(at_sb[:], atps[:])

                # block-one-hot attn stationaries: slab u (cols 32u..32u+31)
                # holds attn for b = 4u+i at within-slab col (u + 8i); the
                # matmul therefore writes b's output to psum row (u + 8i).
                # Global one-hot col = 33u + 8i -> one strided copy per i.
                ohA = gp.tile([128, 32 * NQ], bf16, tag="ohA")
                nc.vector.memset(ohA[:], 0.0)
                ohB = gp.tile([72, 32 * NQ], bf16, tag="ohB")
                nc.vector.memset(ohB[:], 0.0)
                for i in range(4):
                    # b = 4u+i -> at_sb col (b%2)*GRP + b//2 = (i%2)*GRP + 2u + i//2
                    c0 = (i % 2) * GRP + i // 2
                    sa = at_sb[0:128, c0:c0 + 2 * (NQ - 1) + 1:2]
                    da = ohA[:, 8 * i:8 * i + 33 * (NQ - 1) + 1:33]
                    nc.vector.tensor_copy(da, sa)
                    sb_ = at_sb[0:72, 2 * GRP + c0:2 * GRP + c0 + 2 * (NQ - 1) + 1:2]
                    db = ohB[:, 8 * i:8 * i + 33 * (NQ - 1) + 1:33]
                    nc.vector.tensor_copy(db, sb_)

                # V-step: 16 accumulating N=512 matmuls -> whole group output
                vops = ps_tl.tile([GB, 4 * D], f32, tag="tail")
                for u in range(NQ):
                    vqa, vqb = vtiles[u]
                    nc.tensor.matmul(vops[:], ohA[:, 32 * u:32 * u + 32],
                                     vqa[:], start=(u == 0), stop=False,
                                     skip_group_check=True)
                    nc.tensor.matmul(vops[:], ohB[:, 32 * u:32 * u + 32],
                                     vqb[:], start=False, stop=(u == NQ - 1),
                                     skip_group_check=True)

                # psum row u+8i holds b=4u+i at col-block i
                vsb = gp.tile([GB, 4 * D], f32, tag="vsb")
                nc.vector.tensor_copy(vsb[:], vops[:])
                for i in range(4):
                    nc.gpsimd.dma_start(
                        out[b_lo + i:b_lo + i + 4 * (NQ - 1) + 1:4],
                        vsb[8 * i:8 * i + NQ, 128 * i:128 * (i + 1)],
                    )

            # software pipeline: group g's supertile phase overlaps group
            # g-1's softmax/V tail so the TensorEngine never drains.
            prev = None
            for g in range(NG):
                st = emit_group_head(g)
                for u in range(NQ):
                    emit_pair(g, u, st)
                    if u == 2 and prev is not None:
                        emit_tail(prev)
                prev = st
            flush_deferred()
            flush_deferred()
            emit_tail(prev)

    nc.compile()
    return nc


def _prep_host(inputs):
    """Split/relayout the full inputs into 8 per-core input maps."""
    query = np.ascontiguousarray(inputs["query"], dtype=np.float32)
    key = np.ascontiguousarray(inputs["key"], dtype=np.float32)
    val = np.ascontiguousarray(inputs["val"], dtype=np.float32)
    mask = inputs["mask"]
    W0 = np.asarray(inputs["W0"], dtype=np.float32)
    b0 = np.asarray(inputs["b0"], dtype=np.float32)
    a0 = np.asarray(inputs["a0"], dtype=np.float32)
    W1 = np.asarray(inputs["W1"], dtype=np.float32)
    b1 = np.asarray(inputs["b1"], dtype=np.float32)
    a1 = np.asarray(inputs["a1"], dtype=np.float32)
    Wout = np.asarray(inputs["Wout"], dtype=np.float32)
    # bout shifts every unmasked logit equally -> cancels in softmax; unused.

    alpha_const = bool(np.all(a0 == a0[0:1, :]) and np.all(a1 == a1[0:1, :]))

    w0a, w0b, w0c, w0d = W0[0:D], W0[D:2 * D], W0[2 * D:3 * D], W0[3 * D:4 * D]

    woh = np.zeros((H1, 8 * GRP), dtype=np.float32)
    for u in range(GRP // 2):
        woh[0:H2, GRP * u + 2 * u] = Wout[:, 0]
        woh[H2:2 * H2, GRP * u + 2 * u + 1] = Wout[:, 0]

    # selection rows: variant sl routes Q0T rows (2sl, 2sl+1) onto the two
    # T-halves of the supertile's [H1, 2T] layer-0 psum block
    selv = np.zeros((GB, GRP * T2), dtype=np.float32)
    for sl in range(GRP):
        selv[2 * sl, T2 * sl:T2 * sl + T] = 1.0
        selv[2 * sl + 1, T2 * sl + T:T2 * (sl + 1)] = 1.0

    consts = {
        "w0k": (w0b - w0c).astype(bfnp),
        "w0d": w0d.astype(bfnp),
        "w0q": (w0a + w0c).astype(bfnp),
        "w1": W1.astype(bfnp),
        "woh": woh.astype(bfnp),
        "selv": selv.astype(bfnp),
        "b0c": b0.reshape(H1, 1).copy(),
        "b1c": np.concatenate([b1, b1]).reshape(128, 1),
        "id32": np.eye(32, dtype=np.float32).astype(bfnp),
    }
    if alpha_const:
        consts["a0c"] = a0[0].reshape(H1, 1).copy()
        consts["a1c"] = np.concatenate([a1[0], a1[0]]).reshape(128, 1)
    else:
        a0t = np.ascontiguousarray(a0.T)
        a0q = np.ones((H1, 1024), dtype=np.float32)
        a0q[:, 0:T] = a0t
        a0q[:, T:T2] = a0t
        a0q[:, 512:512 + T] = a0t
        a0q[:, 512 + T:512 + T2] = a0t
        consts["a0q"] = a0q
        a1t = np.ascontiguousarray(a1.T)
        consts["a1tp"] = np.concatenate(
            [np.concatenate([a1t, a1t], axis=1)] * 2, axis=0)

    maskadd_full = np.where(mask == 0, np.float32(MASK_PAD), np.float32(0.0))
    maskadd_full = maskadd_full.astype(np.float32)

    in_maps = []
    for c in range(NCORES):
        sl = slice(c * Bs, (c + 1) * Bs)
        m = dict(consts)
        kb = key[sl].astype(bfnp)                     # [Bs, T, D]
        m["key_t2"] = np.ascontiguousarray(
            kb.reshape(NSUP, 2, T, D).transpose(0, 3, 1, 2)).reshape(NSUP, D, T2)
        vb = val[sl].astype(bfnp)
        m["val_q"] = np.ascontiguousarray(
            vb.reshape(Bs // 4, 4, T, D).transpose(0, 2, 1, 3)).reshape(Bs // 4, T, 4 * D)
        qs = np.ascontiguousarray(query[sl].T)
        m["q_t"] = qs
        m["q_tb"] = qs.astype(bfnp)
        m["maskadd"] = np.ascontiguousarray(maskadd_full[sl])
        in_maps.append(m)
    return in_maps, alpha_const


def kernel(**inputs) -> np.ndarray:
    global _last_exec_time_ns, _last_results
    in_maps, alpha_const = _prep_host(inputs)

    ck = ("graph", alpha_const)
    if ck not in _cache:
        _cache[ck] = _build(alpha_const)
    nc = _cache[ck]

    trace = bool(os.environ.get("BASS_KERNEL_TRACE"))
    if trace:
        _install_trace_hook()
    res = run_bass_kernel_spmd(nc, in_maps, core_ids=list(range(NCORES)), trace=trace)
    _last_exec_time_ns = res.exec_time_ns
    _last_results = res
    return np.concatenate([res.results[c]["out"] for c in range(NCORES)], axis=0)
